# revision 1
# baseline (speedup 1.0000x reference)
"""Trainium2 kernel for nn_ContConv1dDense (banded continuous conv with
kernel-MLP), data-parallel over (batch, sequence-half) on 8 NeuronCores.

Math: the reference computes, per (b, i, k in 1..8):
    dt      = (times[b,i] - times[b,i-k]) masked to the band & valid length
    hidden  = relu(dt * W1 + b1)                       # (128,)
    kv      = (hidden @ W2 + b2).reshape(32, 32)       # masked
    out[b,i,:] += features[b,i-k,:] @ kv

For this operator's input family, `times` is sorted along the sequence axis
(so dt >= 0) and b1 == b2 == 0.  Then relu(dt*W1) == dt * max(W1, 0)
exactly, and the whole kernel-MLP collapses to a *constant* 32x32 matrix
V = (max(W1,0) @ W2).reshape(32,32):

    out[b,i,:] = sum_k dt_m[b,i,k] * (features[b,i-k,:] @ V)
               = sum_k dt_m[b,i,k] * FV[b,i-k,:]      with FV = features @ V

This is an exact algebraic identity for those inputs (verified by the guard
below at runtime; a general fallback handles anything else).

Per-core device program (core = 2*b + half, 1024 positions each):
  1. FV = featT.T @ V on the PE (9 small matmuls), bounced to DRAM scratch.
  2. dt tiles [128 pos, 8 k] built from shifted window loads of `times`,
     masked by (i < length) via precomputed iota/length compare and by the
     band mask input.
  3. out tile = sum_k dt[:,k] * FV[i-k] accumulated with fused
     scalar_tensor_tensor ops on the DVE; DMA to output.
"""

import numpy as np

KS = 8          # band width (kernel size)
B = 4
L = 2048
C = 32          # in channels
OUT = 32        # out channels
HALF = 1024     # positions per core
PAD = 8         # halo rows in front of each shard
SEQ = HALF + PAD
NT = HALF // 128  # 8 position-tiles per core
N_CORES = 8

_CACHE = {}


def _build_program():
    from contextlib import ExitStack

    import concourse.bacc as bacc
    import concourse.bass as bass
    import concourse.tile as tile
    from concourse import mybir

    f32 = mybir.dt.float32

    nc = bacc.Bacc(
        "TRN2", target_bir_lowering=False, debug=False, num_devices=N_CORES
    )

    featT = nc.dram_tensor("featT", [C, SEQ], f32, kind="ExternalInput").ap()
    times = nc.dram_tensor("times", [SEQ], f32, kind="ExternalInput").ap()
    leneff = nc.dram_tensor("leneff", [128, 1], f32, kind="ExternalInput").ap()
    band = nc.dram_tensor("band", [128, NT * KS], f32, kind="ExternalInput").ap()
    vmat = nc.dram_tensor("vmat", [C, OUT], f32, kind="ExternalInput").ap()
    iota8 = nc.dram_tensor("iota8", [128, NT], f32, kind="ExternalInput").ap()
    out = nc.dram_tensor("out", [HALF, OUT], f32, kind="ExternalOutput").ap()

    NCH = SEQ // 128 + 1  # 9 FV chunks (last one 8 rows, padded)

    with tile.TileContext(nc) as tc, ExitStack() as ctx:
        singles = ctx.enter_context(tc.tile_pool(name="singles", bufs=1))
        psum = ctx.enter_context(tc.tile_pool(name="psum", bufs=3, space="PSUM"))
        dram = ctx.enter_context(tc.tile_pool(name="dram", bufs=1, space="DRAM"))
        work = ctx.enter_context(tc.tile_pool(name="work", bufs=1))
        pp = ctx.enter_context(tc.tile_pool(name="pp", bufs=2))

        f32r = mybir.dt.float32r
        featT_sb = singles.tile([C, SEQ], f32r)
        nc.gpsimd.dma_start(featT_sb[:], featT[:])
        v_sb = singles.tile([C, OUT], f32r)
        nc.gpsimd.dma_start(v_sb[:], vmat[:])
        len_sb = singles.tile([128, 1], f32)
        nc.sync.dma_start(len_sb[:], leneff[:])
        band_sb = singles.tile([128, NT, KS], f32)
        nc.sync.dma_start(band_sb[:], band[:])
        iota_sb = singles.tile([128, NT], f32)
        nc.sync.dma_start(iota_sb[:], iota8[:])

        # lenmask[p, t] = 1.0 if (128*t + p) < leneff else 0.0
        lenmask = work.tile([128, NT], f32)
        nc.vector.tensor_scalar(
            lenmask[:], iota_sb[:], len_sb[:, 0:1], None, mybir.AluOpType.is_lt
        )

        # t_all[p, t, q] = times[128*t + p + q], q in 0..8; q == 8 is t_i,
        # q < 8 is t_{i-k} with k = 8 - q.
        t_all = work.tile([128, NT, KS + 1], f32)
        nc.sync.dma_start(
            t_all[:],
            bass.AP(tensor=times.tensor, offset=0, ap=[[1, 128], [128, NT], [1, KS + 1]]),
        )
        dt_raw = work.tile([128, NT, KS], f32)
        nc.vector.tensor_tensor(
            dt_raw[:],
            t_all[:, :, KS : KS + 1].to_broadcast([128, NT, KS]),
            t_all[:, :, 0:KS],
            mybir.AluOpType.subtract,
        )
        dt_len = work.tile([128, NT, KS], f32)
        nc.vector.tensor_tensor(
            dt_len[:],
            dt_raw[:],
            lenmask[:, :, None].to_broadcast([128, NT, KS]),
            mybir.AluOpType.mult,
        )
        dt_f = work.tile([128, NT, KS], f32)
        nc.vector.tensor_tensor(dt_f[:], dt_len[:], band_sb[:], mybir.AluOpType.mult)

        # FV[z, :] = featT[:, z].T @ V, staged in SBUF then bounced to DRAM
        # scratch (padded to 9*128 rows) for the banded window gather.
        fv_dram = dram.tile([NCH * 128, OUT], f32)
        fv_stage = work.tile([128, NCH, OUT], f32)
        # last chunk only covers 8 rows; zero-fill so the staging DMA below
        # reads fully initialized memory (pad rows are never consumed).
        nc.vector.memset(fv_stage[:, NCH - 1, :], 0.0)
        for ch in range(NCH):
            z0 = 128 * ch
            csz = min(128, SEQ - z0)
            fv_ps = psum.tile([128, OUT], f32)
            nc.tensor.matmul(
                fv_ps[:csz, :],
                featT_sb[:, z0 : z0 + csz],
                v_sb[:],
                start=True,
                stop=True,
            )
            nc.scalar.copy(fv_stage[:csz, ch, :], fv_ps[:csz, :])
        nc.sync.dma_start(
            bass.AP(
                tensor=fv_dram.tensor,
                offset=fv_dram.offset,
                ap=[[OUT, 128], [128 * OUT, NCH], [1, OUT]],
            ),
            fv_stage[:],
        )

        # Banded window: fv_win[p, t, q, :] = FV[128*t + q + p, :].
        # Processed in halves so multiply/reduce overlaps the gather DMA.
        HT = NT // 2
        for g in range(2):
            t0 = g * HT
            fv_win = pp.tile([128, HT, KS, OUT], f32, tag="fv_win")
            nc.sync.dma_start(
                fv_win[:],
                bass.AP(
                    tensor=fv_dram.tensor,
                    offset=fv_dram.offset + t0 * 128 * OUT,
                    ap=[[OUT, 128], [128 * OUT, HT], [OUT, KS], [1, OUT]],
                ),
            )
            prod = pp.tile([128, HT, KS, OUT], f32, tag="prod")
            nc.vector.tensor_tensor(
                prod[:],
                fv_win[:],
                dt_f[:, t0 : t0 + HT, :, None].to_broadcast([128, HT, KS, OUT]),
                mybir.AluOpType.mult,
            )
            s1 = pp.tile([128, HT, KS // 2, OUT], f32, tag="s1")
            nc.vector.tensor_tensor(
                s1[:], prod[:, :, 0:4, :], prod[:, :, 4:8, :], mybir.AluOpType.add
            )
            s2 = pp.tile([128, HT, KS // 4, OUT], f32, tag="s2")
            nc.vector.tensor_tensor(
                s2[:], s1[:, :, 0:2, :], s1[:, :, 2:4, :], mybir.AluOpType.add
            )
            out_half = pp.tile([128, HT, OUT], f32, tag="out_half")
            nc.vector.tensor_tensor(
                out_half[:], s2[:, :, 0, :], s2[:, :, 1, :], mybir.AluOpType.add
            )
            nc.sync.dma_start(
                bass.AP(
                    tensor=out.tensor,
                    offset=out.offset + t0 * 128 * OUT,
                    ap=[[OUT, 128], [128 * OUT, HT], [1, OUT]],
                ),
                out_half[:],
            )

    nc.compile()
    return nc


def _build_program_raw():
    """Hand-synchronized (non-Tile) variant: same math, minimal sync/teardown
    overhead, pipelined FV-bounce -> per-tile window-gather -> multiply/reduce
    spread across DVE (quarters 1-3) and GpSimd (quarter 0)."""
    from contextlib import ExitStack

    import concourse.bacc as bacc
    import concourse.bass as bass
    from concourse import mybir

    f32 = mybir.dt.float32
    NCH = SEQ // 128 + 1

    nc = bacc.Bacc(
        "TRN2", target_bir_lowering=False, debug=False, num_devices=N_CORES
    )

    pk = nc.dram_tensor("pk", [128, 73], f32, kind="ExternalInput").ap()
    fz = nc.dram_tensor("fz", [C, SEQ + OUT], f32, kind="ExternalInput").ap()
    times = nc.dram_tensor("times", [SEQ], f32, kind="ExternalInput").ap()
    out = nc.dram_tensor("out", [HALF, OUT], f32, kind="ExternalOutput").ap()
    fv_dram = nc.dram_tensor("fv_scratch", [NCH * 128, OUT], f32).ap()

    pk_sb = nc.alloc_sbuf_tensor("pk_sb", [128, 73], f32).ap()
    fz_sb = nc.alloc_sbuf_tensor("fz_sb", [C, SEQ + OUT], f32).ap()
    ta = nc.alloc_sbuf_tensor("ta", [128, NT, KS + 1], f32).ap()
    lm = nc.alloc_sbuf_tensor("lm", [128, NT], f32).ap()
    dtA = nc.alloc_sbuf_tensor("dtA", [128, NT, KS], f32).ap()
    dtB = nc.alloc_sbuf_tensor("dtB", [128, NT, KS], f32).ap()
    fstg = nc.alloc_sbuf_tensor("fstg", [128, NCH, OUT], f32).ap()
    fwin = nc.alloc_sbuf_tensor("fwin", [128, NT, KS, OUT], f32).ap()
    # product staged transposed [p, t, o, q] so one X-axis reduce sums over q
    pr2 = nc.alloc_sbuf_tensor("pr2", [128, NT, OUT, KS], f32).ap()
    # gpsimd quarter uses a plain layout + tree adds (no X-reduce on gpsimd)
    pg = nc.alloc_sbuf_tensor("pg", [128, 2, KS, OUT], f32).ap()
    pg1 = nc.alloc_sbuf_tensor("pg1", [128, 2, KS // 2, OUT], f32).ap()
    pg2 = nc.alloc_sbuf_tensor("pg2", [128, 2, KS // 4, OUT], f32).ap()
    oh = nc.alloc_sbuf_tensor("oh", [128, NT, OUT], f32).ap()
    # one full PSUM bank per buffer so PE writes and ACT reads of
    # consecutive chunks never touch the same bank
    ps = [
        nc.alloc_psum_tensor(f"ps{i}", [128, 512], f32).ap()[:, 0:OUT]
        for i in range(3)
    ]

    band = pk_sb[:, 1:65].rearrange("p (t k) -> p t k", t=NT)
    iota = pk_sb[:, 65:73]
    vsb = fz_sb[:, 0:OUT]

    with ExitStack() as _sctx:
        block = _sctx.enter_context(nc.Block(no_gpsimd_drain=True))
        _names = (
            ["sPK", "sTA", "sFVa", "sFVb", "sMM", "sCP"]
            + [f"sB{i}" for i in range(9)]
            + [f"sT{i}" for i in range(8)]
            + ["sDV", "sD3", "sVD", "sGD", "sOUT"]
        )
        _sems = {n: _sctx.enter_context(nc.semaphore(n)) for n in _names}
        sPK, sTA, sFVa, sFVb = (_sems[n] for n in ("sPK", "sTA", "sFVa", "sFVb"))
        sMM, sCP = _sems["sMM"], _sems["sCP"]
        sDV, sD3, sVD, sGD, sOUT = (
            _sems[n] for n in ("sDV", "sD3", "sVD", "sGD", "sOUT")
        )
        sB = [_sems[f"sB{i}"] for i in range(9)]
        sT = [_sems[f"sT{i}"] for i in range(8)]

        def bounce(raw, ch):
            rows = min(128, SEQ - 128 * ch)
            raw.wait_ge(sCP, ch + 1)
            raw.dma_start(
                bass.AP(
                    tensor=fv_dram.tensor,
                    offset=ch * 128 * OUT,
                    ap=[[OUT, rows], [1, OUT]],
                ),
                fstg[:rows, ch, :],
            ).then_inc(sB[ch], 16)

        def gather(raw, t):
            # tile t covers FV rows [128t, 128t+134] -> chunks t, t+1
            raw.wait_ge(sB[t], 16)
            raw.wait_ge(sB[t + 1], 16)
            raw.dma_start(
                fwin[:, t : t + 1, :, :],
                bass.AP(
                    tensor=fv_dram.tensor,
                    offset=t * 128 * OUT,
                    ap=[[OUT, 128], [OUT, KS], [1, OUT]],
                ),
            ).then_inc(sT[t], 16)

        def out_dma(raw, t):
            raw.dma_start(
                bass.AP(
                    tensor=out.tensor,
                    offset=t * 128 * OUT,
                    ap=[[OUT, 128], [1, OUT]],
                ),
                oh[:, t, :],
            ).then_inc(sOUT, 16)

        def q_mul(eng, raw, g, sself, sv):
            t0 = 2 * g
            raw.wait_ge(sT[t0], 16)
            raw.wait_ge(sT[t0 + 1], 16)
            eng.tensor_tensor(
                bass.AP(
                    tensor=pr2.tensor,
                    offset=t0 * OUT * KS,
                    ap=[[OUT * KS * NT, 128], [OUT * KS, 2], [1, KS], [KS, OUT]],
                ),
                fwin[:, t0 : t0 + 2, :, :],
                dtA[:, t0 : t0 + 2, :, None].to_broadcast([128, 2, KS, OUT]),
                mybir.AluOpType.mult,
            ).then_inc(sself, 1)

        def q_reduce(eng, raw, tt, sself, sv, sdone):
            raw.wait_ge(sself, sv)
            eng.tensor_reduce(
                oh[:, tt, :],
                pr2[:, tt, :, :],
                mybir.AxisListType.X,
                mybir.AluOpType.add,
            ).then_inc(sdone, 1)

        @block.sync
        def _(sync):
            sync.dma_start(fz_sb[:, 0:544], fz[:, 0:544]).then_inc(sFVa, 16)
            sync.dma_start(fz_sb[:, 544:], fz[:, 544:]).then_inc(sFVb, 16)
            sync.dma_start(pk_sb[:], pk[:]).then_inc(sPK, 16)
            sync.dma_start(
                ta[:],
                bass.AP(
                    tensor=times.tensor,
                    offset=0,
                    ap=[[1, 128], [128, NT], [1, KS + 1]],
                ),
            ).then_inc(sTA, 16)
            for ch in (0, 1, 2):
                bounce(sync, ch)
            for t in (0, 1, 2, 3):
                gather(sync, t)
            for sem, n, t in (
                (sDV, 1, 2), (sDV, 2, 3), (sD3, 1, 0), (sD3, 2, 1),
                (sDV, 3, 4), (sDV, 4, 5), (sDV, 5, 6), (sDV, 6, 7),
            ):
                sync.wait_ge(sem, n)
                out_dma(sync, t)
            sync.wait_ge(sOUT, 128)

        @block.gpsimd
        def _(gpsimd):
            for ch in range(3, NCH):
                bounce(gpsimd, ch)
            # quarter 0 of the multiply/reduce runs here, in parallel with
            # the DVE's quarters 1-3
            gpsimd.wait_ge(sVD, 4)  # dtA (masked dt) finalized on the DVE
            gpsimd.wait_ge(sT[0], 16)
            gpsimd.wait_ge(sT[1], 16)
            nc.gpsimd.tensor_tensor(
                pg[:],
                fwin[:, 0:2, :, :],
                dtA[:, 0:2, :, None].to_broadcast([128, 2, KS, OUT]),
                mybir.AluOpType.mult,
            ).then_inc(sGD, 1)
            gpsimd.wait_ge(sGD, 1)
            nc.gpsimd.tensor_tensor(
                pg1[:], pg[:, :, 0:4, :], pg[:, :, 4:8, :], mybir.AluOpType.add
            ).then_inc(sGD, 1)
            gpsimd.wait_ge(sGD, 2)
            nc.gpsimd.tensor_tensor(
                pg2[:], pg1[:, :, 0:2, :], pg1[:, :, 2:4, :], mybir.AluOpType.add
            ).then_inc(sGD, 1)
            gpsimd.wait_ge(sGD, 3)
            nc.gpsimd.tensor_tensor(
                oh[:, 0, :], pg2[:, 0, 0, :], pg2[:, 0, 1, :], mybir.AluOpType.add
            ).then_inc(sD3, 1)
            gpsimd.wait_ge(sGD, 3)
            nc.gpsimd.tensor_tensor(
                oh[:, 1, :], pg2[:, 1, 0, :], pg2[:, 1, 1, :], mybir.AluOpType.add
            ).then_inc(sD3, 1)

        @block.tensor
        def _(tensor):
            tensor.wait_ge(sFVa, 16)
            for ch in range(NCH):
                if ch == 4:
                    tensor.wait_ge(sFVb, 16)
                z0 = 128 * ch
                csz = min(128, SEQ - z0)
                if ch >= 3:
                    tensor.wait_ge(sCP, ch - 2)
                nc.tensor.matmul(
                    ps[ch % 3][:csz, :],
                    fz_sb[:, OUT + z0 : OUT + z0 + csz],
                    vsb,
                    start=True,
                    stop=True,
                ).then_inc(sMM, 1)

        @block.scalar
        def _(scalar):
            for ch in range(NCH):
                csz = min(128, SEQ - 128 * ch)
                scalar.wait_ge(sMM, ch + 1)
                nc.scalar.copy(fstg[:csz, ch, :], ps[ch % 3][:csz, :]).then_inc(
                    sCP, 1
                )
            for t in (4, 5, 6, 7):
                gather(scalar, t)

        @block.vector
        def _(vector):
            vector.wait_ge(sPK, 16)
            nc.vector.tensor_scalar(
                lm[:], iota, pk_sb[:, 0:1], None, mybir.AluOpType.is_lt
            ).then_inc(sVD, 1)
            vector.wait_ge(sTA, 16)
            nc.vector.tensor_tensor(
                dtA[:],
                ta[:, :, KS : KS + 1].to_broadcast([128, NT, KS]),
                ta[:, :, 0:KS],
                mybir.AluOpType.subtract,
            ).then_inc(sVD, 1)
            vector.wait_ge(sVD, 2)
            nc.vector.tensor_tensor(
                dtB[:],
                dtA[:],
                lm[:, :, None].to_broadcast([128, NT, KS]),
                mybir.AluOpType.mult,
            ).then_inc(sVD, 1)
            vector.wait_ge(sVD, 3)
            nc.vector.tensor_tensor(
                dtA[:], dtB[:], band, mybir.AluOpType.mult
            ).then_inc(sVD, 1)
            vector.wait_ge(sVD, 4)
            for g in (1, 2, 3):
                q_mul(nc.vector, vector, g, sVD, None)
                sv = 4 + g
                q_reduce(nc.vector, vector, 2 * g, sVD, sv, sDV)
                q_reduce(nc.vector, vector, 2 * g + 1, sVD, sv, sDV)

    nc.compile()
    return nc


def _get_program():
    if "nc" not in _CACHE:
        _CACHE["nc"] = _build_program_raw()
    return _CACHE["nc"]


def _fast_path_ok(times, b1, b2):
    # The linearization relu(dt*W1 + b1) == dt*max(W1,0) is exact iff
    # b1 == 0 and dt >= 0 (times sorted); b2 == 0 removes the bias term.
    if np.any(b1 != 0.0) or np.any(b2 != 0.0):
        return False
    if np.any(np.diff(times, axis=1) < 0.0):
        return False
    return True


def _reference_fallback(times, features, lengths, W1, b1, W2, b2):
    # Straight numpy transcription of the reference (general inputs).
    Bn, Ln = times.shape
    offsets = np.arange(1, KS + 1)
    idx = np.arange(Ln)[:, None] - offsets[None, :]
    in_band = idx >= 0
    idx_c = np.clip(idx, 0, Ln - 1)
    t_j = times[:, idx_c]
    dt = times[:, :, None] - t_j
    pos_i = np.arange(Ln)[None, :, None]
    mask = (
        in_band[None]
        & (idx_c[None] < lengths[:, None, None])
        & (pos_i <= lengths[:, None, None] - 1)
    )
    dt = np.where(mask, dt, 0.0).astype(np.float32)
    hidden = np.maximum(dt[..., None] * W1[0] + b1, 0.0)
    kv = (hidden @ W2 + b2).reshape(Bn, Ln, KS, C, OUT)
    kv = np.where(mask[..., None, None], kv, 0.0)
    feat_g = features[:, idx_c]
    return np.einsum("blkc,blkco->blo", feat_g, kv).astype(np.float32)


def _build_in_maps(times, features, lengths, W1, W2):
    # Fold the (now linear) kernel-MLP into one 32x32 matrix.
    v = (np.maximum(W1[0], 0.0) @ W2).reshape(C, OUT).astype(np.float32)
    iota8 = (
        np.arange(128, dtype=np.float32)[:, None]
        + 128.0 * np.arange(NT, dtype=np.float32)[None, :]
    ).astype(np.float32)

    in_maps = []
    for core in range(N_CORES):
        b, half = core // 2, core % 2
        start = half * HALF
        ft = np.zeros((SEQ, C), np.float32)
        tm = np.empty((SEQ,), np.float32)
        lo = start - PAD
        if lo < 0:
            ft[PAD:] = features[b, 0:start + HALF]
            tm[:PAD] = times[b, 0]
            tm[PAD:] = times[b, 0:start + HALF]
        else:
            ft[:] = features[b, lo:start + HALF]
            tm[:] = times[b, lo:start + HALF]
        leneff = np.full((128, 1), float(lengths[b]) - start, np.float32)
        # band[p, t, q] = 1 if global position (start+128t+p) >= k (= 8-q);
        # only the first 8 rows of tile 0 can violate the band.
        p = (
            np.arange(128)[:, None, None]
            + 128 * np.arange(NT)[None, :, None]
            + start
        )
        k = KS - np.arange(KS)[None, None, :]
        bandm = np.ascontiguousarray(
            (p >= k).astype(np.float32).reshape(128, NT * KS)
        )
        pk = np.empty((128, 73), np.float32)
        pk[:, 0:1] = leneff
        pk[:, 1:65] = bandm
        pk[:, 65:73] = iota8
        fz = np.empty((C, SEQ + OUT), np.float32)
        fz[:, :OUT] = v
        fz[:, OUT:] = ft.T
        in_maps.append({"pk": pk, "fz": fz, "times": tm})
    return in_maps


def kernel(times, features, lengths, W1, b1, W2, b2):
    times = np.asarray(times, dtype=np.float32)
    features = np.asarray(features, dtype=np.float32)
    lengths = np.asarray(lengths)
    W1 = np.asarray(W1, dtype=np.float32)
    b1 = np.asarray(b1, dtype=np.float32)
    W2 = np.asarray(W2, dtype=np.float32)
    b2 = np.asarray(b2, dtype=np.float32)

    if not _fast_path_ok(times, b1, b2):
        return _reference_fallback(times, features, lengths, W1, b1, W2, b2)

    from concourse.bass_utils import run_bass_kernel_spmd

    nc = _get_program()
    in_maps = _build_in_maps(times, features, lengths, W1, W2)
    res = run_bass_kernel_spmd(nc, in_maps, core_ids=list(range(N_CORES)))

    out = np.empty((B, L, OUT), np.float32)
    for core in range(N_CORES):
        b, half = core // 2, core % 2
        out[b, half * HALF : (half + 1) * HALF, :] = res.results[core]["out"]
    return out

